# revision 7
# baseline (speedup 1.0000x reference)
"""ArcFace loss on 8 TRN2 NeuronCores (class-dim tensor parallel).

Strategy
--------
Device (per core, classes sharded 8 x 12500):
    cos[n, c] = e_norm[n, :] @ w_norm[c, :]^T   (fp8 DoubleRow in, f32 PSUM)
    partial[n, g] = sum_{c in group g} exp(64 * cos[n, c])
  via TensorE matmuls into [128, 2048] PSUM groups and a single ScalarE
  activation(Exp, scale, accum_out=...) per group that performs the
  exp AND the row-sum in one instruction.

fp8 path: operands are L2-normalized rows pre-scaled by 16 and cast to
float8_e4m3 on the host; DoubleRow packs 2 contraction planes per PE cell
(contraction 256/matmul, ~1.5x bf16 rate).  logits = dot/(16*16)*64 =
dot * 0.25 which becomes the activation scale.  Class dim is padded
12500 -> 12512 per core (plane stride must be %16); each zero pad column
contributes exp(0)=1, subtracted exactly on the host (96 total).

Host:
  - computes the target-class terms exactly in f64 (cos_gt, margin logit,
    keep mask -- only 1024 rows, negligible)
  - combines the 8x[1024, 7] device partials in f64:
        S = sum partials - pads; S_corr = S - exp(64*cos_gt) + exp(64*cos_margin)
        nll = log(S_corr) - 64*cos_margin;  loss = masked mean
  The margin only modifies the single target column per row, so the device
  computes the plain (unmargined) sum-of-exp and the host swaps in the
  margined target term.  No max-subtraction is needed: |cos| <= 1 so
  logits are in [-64, 64] and sums stay far below f32/f64 overflow.
"""

import numpy as np

N, E, C = 1024, 512, 100000
M = 8            # cores
CSH = C // M     # 12500 classes per core
P = 128
NT = N // P      # 8 batch-row tiles
SCALE = 64.0
MARGIN = 0.5
EPS_NORM = 1e-12
CLAMP = 1e-7

USE_FP8 = True
FP8_S = 16.0                   # pre-scale on both operands
CSHP = 12512 if USE_FP8 else 12500   # padded class columns per core
GROUPS = [2048] * 6 + [CSHP - 2048 * 6]   # PSUM-group widths (sum = CSHP)
NG = len(GROUPS)
KP = 2 if USE_FP8 else 4       # contraction chunks (256 per DoubleRow mm, else 128)

_compiled = None
LAST_RESULT = None  # BassKernelResults of the most recent run (for test.py)


def _np_in_dtype():
    import concourse.mybir as mybir
    return mybir.dt.np(mybir.dt.float8e4 if USE_FP8 else mybir.dt.float16)


def _build(reps=None):
    import contextlib

    import concourse.mybir as mybir
    import concourse.tile as tile
    from concourse import bacc

    f32 = mybir.dt.float32
    fin = mybir.dt.float8e4 if USE_FP8 else mybir.dt.float16
    EXP = mybir.ActivationFunctionType.Exp
    act_scale = SCALE / (FP8_S * FP8_S) if USE_FP8 else SCALE
    perf = mybir.MatmulPerfMode.DoubleRow if USE_FP8 else None
    # tile free layout: [2, width] plane pair for DoubleRow, [width] otherwise
    KSUB = 2 if USE_FP8 else 1

    nc = bacc.Bacc("TRN2", target_bir_lowering=False, debug=False, num_devices=M)
    et_d = nc.dram_tensor("et", [KP, P, KSUB, N], fin, kind="ExternalInput").ap()
    wt_d = nc.dram_tensor("wt", [KP, P, KSUB, CSHP], fin, kind="ExternalInput").ap()
    out_d = nc.dram_tensor("out", [N, NG], f32, kind="ExternalOutput").ap()

    with tile.TileContext(nc) as tc:
        with tc.tile_pool(name="wp", bufs=1) as wp, \
             tc.tile_pool(name="ep", bufs=1) as ep, \
             tc.tile_pool(name="sp", bufs=1) as sp, \
             tc.tile_pool(name="ps", bufs=2, space="PSUM") as pp, \
             (tc.For_i(0, reps, 1) if reps else contextlib.nullcontext()):
            # dummy exp with no deps: walrus places ACT_TABLE_LOAD before the
            # first Exp ACTIVATE, so this overlaps the ~2.7us table load with
            # the weight DMAs instead of paying it on the critical path
            warm = ep.tile([P, 1], f32, tag="warm", name="warm")
            nc.vector.memset(warm[:], 0.0)
            warm2 = ep.tile([P, 1], f32, tag="warm2", name="warm2")
            nc.scalar.activation(warm2[:], warm[:], EXP, scale=1.0)

            et = []
            for k in range(KP):
                t = ep.tile([P, KSUB, N], fin, tag=f"et{k}", name=f"et{k}")
                nc.sync.dma_start(t[:], et_d[k])
                et.append(t)
            stats = [sp.tile([P, NG], f32, tag=f"st{t}", name=f"st{t}") for t in range(NT)]
            # whole weight shard is SBUF resident; one tile per (group, k) so
            # compute can start as soon as group 0 lands
            wt = []
            col = 0
            for g, gw in enumerate(GROUPS):
                tk = []
                for k in range(KP):
                    t = wp.tile([P, KSUB, gw], fin, tag=f"w{g}_{k}", name=f"w{g}_{k}")
                    nc.sync.dma_start(t[:], wt_d[k, :, :, col:col + gw])
                    tk.append(t)
                wt.append(tk)
                col += gw

            for g, gw in enumerate(GROUPS):
                nsub = (gw + 511) // 512
                for t in range(NT):
                    ps = pp.tile([P, 2048], f32, tag="ps", name=f"ps{g}_{t}")
                    # k outer / j inner: consecutive matmuls keep the same
                    # stationary operand and rotate PSUM banks
                    for k in range(KP):
                        for j in range(nsub):
                            jw = min(512, gw - j * 512)
                            nc.tensor.matmul(
                                ps[:, j * 512:j * 512 + jw],
                                et[k][:, :, t * P:(t + 1) * P],
                                wt[g][k][:, :, j * 512:j * 512 + jw],
                                start=(k == 0),
                                stop=(k == KP - 1),
                                perf_mode=perf,
                            )
                    # exp results are scratch (only the accum row-sum matters);
                    # in-place PSUM read+write faults the HW, so write to SBUF
                    ex = ep.tile([P, 2048], f32, tag="ex", name=f"ex{g}_{t}")
                    nc.scalar.activation(
                        ex[:, :gw], ps[:, :gw], EXP,
                        scale=act_scale,
                        accum_out=stats[t][:, g:g + 1],
                    )

            for t in range(NT):
                nc.sync.dma_start(out_d[t * P:(t + 1) * P, :], stats[t][:])

    nc.compile()
    return nc


def _prep_operands(e, w):
    """Normalize rows, pre-scale, quantize, and lay out [KP, P, KSUB, cols]."""
    dt = _np_in_dtype()
    s = FP8_S if USE_FP8 else 1.0
    wn = (w * (s / np.maximum(np.sqrt(np.einsum('ij,ij->i', w, w)), EPS_NORM))[:, None]).astype(dt)
    en = (e * (s / np.maximum(np.sqrt(np.einsum('ij,ij->i', e, e)), EPS_NORM))[:, None]).astype(dt)

    def lay(xT, cols):  # xT: [E, cols] -> [KP, P, KSUB, cols]
        ksub = 2 if USE_FP8 else 1
        return np.ascontiguousarray(
            xT.reshape(KP, ksub, P, cols).transpose(0, 2, 1, 3))

    et_arr = lay(np.ascontiguousarray(en.T), N)
    shards = []
    for i in range(M):
        blk = wn[i * CSH:(i + 1) * CSH]
        bT = np.zeros((E, CSHP), dt)
        bT[:, :CSH] = blk.T
        shards.append(lay(bT, CSHP))
    return et_arr, shards


def kernel(embedding, ground_truth, weight):
    global _compiled, LAST_RESULT
    from concourse.bass_utils import run_bass_kernel_spmd

    e = np.ascontiguousarray(np.asarray(embedding, dtype=np.float32))
    w = np.ascontiguousarray(np.asarray(weight, dtype=np.float32))
    gt = np.asarray(ground_truth).astype(np.int64)

    et_arr, shards = _prep_operands(e, w)
    in_maps = [{"et": et_arr, "wt": shards[i]} for i in range(M)]

    if _compiled is None:
        _compiled = _build()
    LAST_RESULT = run_bass_kernel_spmd(_compiled, in_maps, core_ids=list(range(M)))

    # ---- host combine (f64) ----
    S = np.zeros(N, np.float64)
    for r in LAST_RESULT.results:
        S += r["out"].astype(np.float64).sum(axis=1)
    S -= float(M * (CSHP - CSH))   # zero-pad columns contribute exp(0)=1 each

    # exact target-class terms
    e64 = e.astype(np.float64)
    en64 = e64 / np.maximum(np.sqrt((e64 * e64).sum(1, keepdims=True)), EPS_NORM)
    wg = w[gt].astype(np.float64)
    wg /= np.maximum(np.sqrt((wg * wg).sum(1, keepdims=True)), EPS_NORM)
    cos_gt = np.clip((en64 * wg).sum(1), -1.0 + CLAMP, 1.0 - CLAMP)
    keep = (np.arccos(cos_gt) + MARGIN) <= np.pi
    tgt = SCALE * (cos_gt * np.cos(MARGIN) - np.sqrt(1.0 - cos_gt * cos_gt) * np.sin(MARGIN))

    S_corr = S - np.exp(SCALE * cos_gt) + np.exp(tgt)
    nll = np.log(S_corr) - tgt
    loss = (nll * keep).sum() / max(keep.sum(), 1.0)
    return np.float32(loss)
